# revision 11
# baseline (speedup 1.0000x reference)
"""Trainium2 Bass kernel for CumulativeSetAttentionLayer (segment_reduce).

Strategy (8 NeuronCores, data-parallel over tokens):
  - Shard the 131072 tokens across 8 cores at segment boundaries (the
    cumulative-segment-mean never crosses a core); pad each shard to a
    common NPAD so one SPMD program serves all cores.
  - Feature-major layout on device: activations live as [feature, token]
    tiles so every MLP layer is a plain PE matmul with the weight as the
    stationary operand and tokens as the moving free dim. The input is
    transposed once on the host.
  - The cumulative segment mean is a first-order linear recurrence
    y[t] = gamma[t]*y[t-1] + x[t] (gamma=0 at segment starts), computed by
    the DVE tensor_tensor_scan along the token axis; the 1/count scale is
    applied after the rho matmul (it commutes: rho mixes features only).
  - keys @ W_q collapses into a single [640, 8] matrix on the host:
    preattn = [inputs, agg] @ fold where fold[c,h] = sum_d W_k[c,h,d]*W_q[h,d].
  - Matmuls run in float32r (4x fp32 PE throughput, ~1.5e-4 rel err).
"""

import math

import numpy as np

import concourse.bacc as bacc
import concourse.tile as tile
from concourse import mybir
from concourse.bass_utils import run_bass_kernel_spmd

N, D_IN, WID, LAT, DPD, H = 131072, 128, 512, 512, 128, 8
NCORES = 8
F = 512  # tokens per chunk = matmul moving dim = one PSUM bank of fp32
NB = WID // 128  # feature blocks per 512-wide layer

F32 = mybir.dt.float32
F32R = mybir.dt.float32r
RELU = mybir.ActivationFunctionType.Relu

_BUILD_CACHE = {}


def _build(npad):
    nchunk = npad // F
    nc = bacc.Bacc(None, target_bir_lowering=False, debug=False)

    xT = nc.dram_tensor("xT", [D_IN, npad], F32, kind="ExternalInput")
    gam = nc.dram_tensor("gam", [1, npad], F32, kind="ExternalInput")
    invc = nc.dram_tensor("invc", [1, npad], F32, kind="ExternalInput")
    w1 = nc.dram_tensor("w1", [D_IN, WID], F32, kind="ExternalInput")
    w2 = nc.dram_tensor("w2", [WID, WID], F32, kind="ExternalInput")
    w3 = nc.dram_tensor("w3", [WID, LAT], F32, kind="ExternalInput")
    wr = nc.dram_tensor("wr", [LAT, LAT], F32, kind="ExternalInput")
    wfx = nc.dram_tensor("wfx", [D_IN, H], F32, kind="ExternalInput")
    wfa = nc.dram_tensor("wfa", [LAT, H], F32, kind="ExternalInput")
    bias = nc.dram_tensor("bias", [128, 4 * NB], F32, kind="ExternalInput")
    out = nc.dram_tensor("out", [H, npad], F32, kind="ExternalOutput")

    with tile.TileContext(nc) as tc:
        with (
            tc.tile_pool(name="wpool", bufs=1) as wpool,
            tc.tile_pool(name="stage", bufs=2) as stage,
            tc.tile_pool(name="io", bufs=4) as io,
            tc.tile_pool(name="acts", bufs=2) as acts,
            tc.tile_pool(name="scan", bufs=3) as scan,
            tc.tile_pool(name="psum", bufs=6, space="PSUM") as psum,
            tc.tile_pool(name="psum8", bufs=2, space="PSUM") as psum8,
        ):
            # ---- resident weights, rounded to fp32r once ----
            w1s = wpool.tile([128, WID], F32R)
            t = stage.tile([128, WID], F32, tag="wstage")
            nc.sync.dma_start(out=t, in_=w1[:, :])
            nc.vector.tensor_copy(w1s, t)

            def load_big(dram):
                s = wpool.tile([128, NB, WID], F32R, tag=f"w_{dram.name}")
                for kb in range(NB):
                    tt = stage.tile([128, WID], F32, tag="wstage")
                    nc.sync.dma_start(out=tt, in_=dram[kb * 128 : (kb + 1) * 128, :])
                    nc.vector.tensor_copy(s[:, kb, :], tt)
                return s

            w2s = load_big(w2)
            w3s = load_big(w3)
            wrs = load_big(wr)

            wfxs = wpool.tile([128, H], F32R)
            t = stage.tile([128, H], F32, tag="wfstage")
            nc.sync.dma_start(out=t, in_=wfx[:, :])
            nc.vector.tensor_copy(wfxs, t)
            wfas = wpool.tile([128, NB, H], F32R)
            for kb in range(NB):
                t = stage.tile([128, H], F32, tag="wfstage")
                nc.sync.dma_start(out=t, in_=wfa[kb * 128 : (kb + 1) * 128, :])
                nc.vector.tensor_copy(wfas[:, kb, :], t)

            bs = wpool.tile([128, 4 * NB], F32)
            nc.sync.dma_start(out=bs, in_=bias[:, :])
            b1s, b2s, b3s, brs = (bs[:, i * NB : (i + 1) * NB] for i in range(4))

            zcarry = wpool.tile([128, NB], F32)
            nc.vector.memset(zcarry, 0.0)

            def emit_rho(st):
                # rho: agg = relu((y/count) @ Wr + br) ; 1/count applied post-matmul
                y, it = st["y"], st["it"]
                agg = scan.tile([128, NB, F], F32R, tag="agg")
                for mb in range(NB):
                    ps = psum.tile([128, F], F32, tag="ps")
                    for kb in range(NB):
                        nc.tensor.matmul(
                            ps[:, :], wrs[:, kb, mb * 128 : (mb + 1) * 128], y[:, kb, :],
                            start=(kb == 0), stop=(kb == NB - 1),
                        )
                    sc = io.tile([128, F], F32, tag="sc")
                    nc.vector.tensor_tensor(
                        out=sc[:, :], in0=ps[:, :], in1=it[:, :], op=mybir.AluOpType.mult
                    )
                    nc.scalar.activation(agg[:, mb, :], sc, RELU, bias=brs[:, mb : mb + 1])
                st["agg"] = agg

            def emit_fold(st):
                # preattn = [x, agg] @ fold  -> [8, F]
                agg, xr = st["agg"], st["xr"]
                pf = psum8.tile([H, F], F32)
                nc.tensor.matmul(pf[:, :], wfxs[:, :], xr[:, :], start=True, stop=False)
                for kb in range(NB):
                    nc.tensor.matmul(
                        pf[:, :], wfas[:, kb, :], agg[:, kb, :],
                        start=False, stop=(kb == NB - 1),
                    )
                oc = io.tile([H, F], F32, tag="oc")
                nc.vector.tensor_copy(oc, pf)
                nc.sync.dma_start(out=out[:, st["cs"]], in_=oc)

            prev_y = None
            p1 = p2 = None
            for c in range(nchunk):
                cs = slice(c * F, (c + 1) * F)

                xf = io.tile([128, F], F32)
                nc.sync.dma_start(out=xf, in_=xT[:, cs])
                xr = io.tile([128, F], F32R)
                nc.vector.tensor_copy(xr, xf)
                gt = io.tile([128, F], F32)
                nc.sync.dma_start(out=gt, in_=gam[0:1, cs].partition_broadcast(128))
                it = io.tile([128, F], F32)
                nc.sync.dma_start(out=it, in_=invc[0:1, cs].partition_broadcast(128))

                # psi layer 1: [128, F] -> [512, F]
                h1 = acts.tile([128, NB, F], F32R)
                for mb in range(NB):
                    ps = psum.tile([128, F], F32, tag="ps")
                    nc.tensor.matmul(
                        ps[:, :], w1s[:, mb * 128 : (mb + 1) * 128], xr[:, :],
                        start=True, stop=True,
                    )
                    nc.scalar.activation(h1[:, mb, :], ps, RELU, bias=b1s[:, mb : mb + 1])

                # psi layer 2
                h2 = acts.tile([128, NB, F], F32R)
                for mb in range(NB):
                    ps = psum.tile([128, F], F32, tag="ps")
                    for kb in range(NB):
                        nc.tensor.matmul(
                            ps[:, :], w2s[:, kb, mb * 128 : (mb + 1) * 128], h1[:, kb, :],
                            start=(kb == 0), stop=(kb == NB - 1),
                        )
                    nc.scalar.activation(h2[:, mb, :], ps, RELU, bias=b2s[:, mb : mb + 1])

                # psi latent projection
                enc = acts.tile([128, NB, F], F32R)
                for mb in range(NB):
                    ps = psum.tile([128, F], F32, tag="ps")
                    for kb in range(NB):
                        nc.tensor.matmul(
                            ps[:, :], w3s[:, kb, mb * 128 : (mb + 1) * 128], h2[:, kb, :],
                            start=(kb == 0), stop=(kb == NB - 1),
                        )
                    nc.scalar.activation(enc[:, mb, :], ps, RELU, bias=b3s[:, mb : mb + 1])

                # segmented cumulative sum along tokens (resets where gamma=0)
                y = scan.tile([128, NB, F], F32R)
                for b in range(NB):
                    init = zcarry[:, b : b + 1] if prev_y is None else prev_y[:, b, F - 1 : F]
                    nc.vector.tensor_tensor_scan(
                        out=y[:, b, :], data0=gt[:, :], data1=enc[:, b, :].bitcast(F32),
                        initial=init, op0=mybir.AluOpType.mult, op1=mybir.AluOpType.add,
                    )
                prev_y = y

                # 3-stage software pipeline: rho lags one chunk, fold lags two,
                # so the PE never queues a matmul whose operand (y from the DVE
                # scan, agg from the rho->scale->relu chain) isn't ready yet.
                if p1 is not None:
                    emit_rho(p1)
                if p2 is not None:
                    emit_fold(p2)
                p2, p1 = p1, {"y": y, "it": it, "xr": xr, "cs": cs}
            emit_rho(p1)
            if p2 is not None:
                emit_fold(p2)
            emit_fold(p1)

    nc.compile()
    return nc


def kernel(inputs, segment_ids, W1, b1, W2, b2, W3, b3, Wr, br, W_k, W_q):
    inputs = np.ascontiguousarray(inputs, np.float32)
    segment_ids = np.asarray(segment_ids)
    n = inputs.shape[0]

    # fold the per-head attention weights: preattn = [inputs, agg] @ wfold
    wfold = np.einsum(
        "chd,hd->ch",
        np.asarray(W_k, np.float64).reshape(D_IN + LAT, H, DPD),
        np.asarray(W_q, np.float64),
    ) / math.sqrt(DPD)
    wfold = wfold.astype(np.float32)
    wfx_np = np.ascontiguousarray(wfold[:D_IN])
    wfa_np = np.ascontiguousarray(wfold[D_IN:])

    # shard at segment boundaries nearest the N/8 grid
    is_start = np.empty(n, bool)
    is_start[0] = True
    is_start[1:] = segment_ids[1:] != segment_ids[:-1]
    starts = np.flatnonzero(is_start)
    bounds = [0]
    for c in range(1, NCORES):
        ideal = c * (n // NCORES)
        i = np.searchsorted(starts, ideal)
        cand = int(starts[i]) if i < len(starts) else n
        if i > 0 and abs(int(starts[i - 1]) - ideal) <= abs(cand - ideal):
            cand = int(starts[i - 1])
        bounds.append(max(cand, bounds[-1]))
    bounds.append(n)

    maxlen = max(bounds[c + 1] - bounds[c] for c in range(NCORES))
    npad = ((maxlen + F - 1) // F) * F

    if npad not in _BUILD_CACHE:
        _BUILD_CACHE[npad] = _build(npad)
    nc = _BUILD_CACHE[npad]

    bias_np = np.stack(
        [np.asarray(v, np.float32).reshape(NB, 128).T for v in (b1, b2, b3, br)], 1
    ).reshape(128, 4 * NB)
    bias_np = np.ascontiguousarray(bias_np)

    common = {
        "w1": np.ascontiguousarray(W1, np.float32),
        "w2": np.ascontiguousarray(W2, np.float32),
        "w3": np.ascontiguousarray(W3, np.float32),
        "wr": np.ascontiguousarray(Wr, np.float32),
        "wfx": wfx_np,
        "wfa": wfa_np,
        "bias": bias_np,
    }

    in_maps = []
    for c in range(NCORES):
        lo, hi = bounds[c], bounds[c + 1]
        L = hi - lo
        xT = np.zeros((D_IN, npad), np.float32)
        xT[:, :L] = inputs[lo:hi].T
        g = np.ones((1, npad), np.float32)
        g[0, :L] = 1.0 - is_start[lo:hi]
        g[0, 0] = 0.0
        if L < npad:
            g[0, L] = 0.0
        idx = np.arange(L)
        st = np.maximum.accumulate(np.where(is_start[lo:hi], idx, 0))
        iv = np.ones((1, npad), np.float32)
        iv[0, :L] = 1.0 / (idx - st + 1.0)
        in_maps.append({"xT": xT, "gam": g, "invc": iv, **common})

    global _LAST_IN_MAPS
    _LAST_IN_MAPS = in_maps

    res = run_bass_kernel_spmd(nc, in_maps, core_ids=list(range(NCORES)))

    full = np.empty((n, H), np.float32)
    for c in range(NCORES):
        lo, hi = bounds[c], bounds[c + 1]
        full[lo:hi] = res.results[c]["out"][:, : hi - lo].T
    return full


# revision 14
# speedup vs baseline: 1.3435x; 1.3435x over previous
"""Trainium2 Bass kernel for CumulativeSetAttentionLayer (segment_reduce).

Strategy (8 NeuronCores, data-parallel over tokens):
  - Shard the 131072 tokens across 8 cores at segment boundaries (the
    cumulative-segment-mean never crosses a core); pad each shard to a
    common NPAD so one SPMD program serves all cores.
  - Feature-major layout on device: activations live as [feature, token]
    tiles so every MLP layer is a plain PE matmul with the weight as the
    stationary operand and tokens as the moving free dim. The input is
    transposed once on the host.
  - The cumulative segment mean is a first-order linear recurrence
    y[t] = gamma[t]*y[t-1] + x[t] (gamma=0 at segment starts), computed by
    the DVE tensor_tensor_scan along the token axis; the 1/count scale is
    applied after the rho matmul (it commutes: rho mixes features only).
  - keys @ W_q collapses into a single [640, 8] matrix on the host:
    preattn = [inputs, agg] @ fold where fold[c,h] = sum_d W_k[c,h,d]*W_q[h,d].
  - Matmuls run in float32r (4x fp32 PE throughput, ~1.5e-4 rel err).
"""

import math

import numpy as np

import concourse.bacc as bacc
import concourse.tile as tile
from concourse import mybir
from concourse.bass_utils import run_bass_kernel_spmd

N, D_IN, WID, LAT, DPD, H = 131072, 128, 512, 512, 128, 8
NCORES = 8
F = 512  # tokens per chunk = matmul moving dim = one PSUM bank of fp32
NB = WID // 128  # feature blocks per 512-wide layer

F32 = mybir.dt.float32
F32R = mybir.dt.float32r
RELU = mybir.ActivationFunctionType.Relu

_BUILD_CACHE = {}


def _build(npad):
    nchunk = npad // F
    nc = bacc.Bacc(None, target_bir_lowering=False, debug=False)

    xT = nc.dram_tensor("xT", [D_IN, npad], F32, kind="ExternalInput")
    gam = nc.dram_tensor("gam", [1, npad], F32, kind="ExternalInput")
    invc = nc.dram_tensor("invc", [1, npad], F32, kind="ExternalInput")
    w1 = nc.dram_tensor("w1", [D_IN, WID], F32, kind="ExternalInput")
    w2 = nc.dram_tensor("w2", [WID, WID], F32, kind="ExternalInput")
    w3 = nc.dram_tensor("w3", [WID, LAT], F32, kind="ExternalInput")
    wr = nc.dram_tensor("wr", [LAT, LAT], F32, kind="ExternalInput")
    wfx = nc.dram_tensor("wfx", [D_IN, H], F32, kind="ExternalInput")
    wfa = nc.dram_tensor("wfa", [LAT, H], F32, kind="ExternalInput")
    bias = nc.dram_tensor("bias", [128, 4 * NB], F32, kind="ExternalInput")
    out = nc.dram_tensor("out", [H, npad], F32, kind="ExternalOutput")

    with tile.TileContext(nc) as tc:
        with (
            tc.tile_pool(name="wpool", bufs=1) as wpool,
            tc.tile_pool(name="stage", bufs=2) as stage,
            tc.tile_pool(name="io", bufs=3) as io,
            tc.tile_pool(name="acts", bufs=2) as acts,
            tc.tile_pool(name="scan", bufs=3) as scan,
            tc.tile_pool(name="psum", bufs=6, space="PSUM") as psum,
            tc.tile_pool(name="psum8", bufs=2, space="PSUM") as psum8,
        ):
            # ---- resident weights, rounded to fp32r once ----
            w1s = wpool.tile([128, WID], F32R)
            t = stage.tile([128, WID], F32, tag="wstage")
            nc.sync.dma_start(out=t, in_=w1[:, :])
            nc.vector.tensor_copy(w1s, t)

            def load_big(dram):
                s = wpool.tile([128, NB, WID], F32R, tag=f"w_{dram.name}")
                for kb in range(NB):
                    tt = stage.tile([128, WID], F32, tag="wstage")
                    nc.sync.dma_start(out=tt, in_=dram[kb * 128 : (kb + 1) * 128, :])
                    nc.vector.tensor_copy(s[:, kb, :], tt)
                return s

            w2s = load_big(w2)
            w3s = load_big(w3)
            wrs = load_big(wr)

            wfxs = wpool.tile([128, H], F32R)
            t = stage.tile([128, H], F32, tag="wfstage")
            nc.sync.dma_start(out=t, in_=wfx[:, :])
            nc.vector.tensor_copy(wfxs, t)
            wfas = wpool.tile([128, NB, H], F32R)
            for kb in range(NB):
                t = stage.tile([128, H], F32, tag="wfstage")
                nc.sync.dma_start(out=t, in_=wfa[kb * 128 : (kb + 1) * 128, :])
                nc.vector.tensor_copy(wfas[:, kb, :], t)

            bs = wpool.tile([128, 4 * NB], F32)
            nc.sync.dma_start(out=bs, in_=bias[:, :])
            b1s, b2s, b3s, brs = (bs[:, i * NB : (i + 1) * NB] for i in range(4))

            zcarry = wpool.tile([128, NB], F32)
            nc.vector.memset(zcarry, 0.0)

            def emit_rho(st):
                # rho: agg = relu((y/count) @ Wr + br) ; 1/count applied post-matmul
                y, it = st["y"], st["it"]
                agg = scan.tile([128, NB, F], F32R, tag="agg")
                for mb in range(NB):
                    ps = psum.tile([128, F], F32, tag="ps")
                    for kb in range(NB):
                        nc.tensor.matmul(
                            ps[:, :], wrs[:, kb, mb * 128 : (mb + 1) * 128], y[:, kb, :],
                            start=(kb == 0), stop=(kb == NB - 1),
                        )
                    sc = io.tile([128, F], F32, tag="sc")
                    nc.vector.tensor_tensor(
                        out=sc[:, :], in0=ps[:, :], in1=it[:, :], op=mybir.AluOpType.mult
                    )
                    nc.scalar.activation(agg[:, mb, :], sc, RELU, bias=brs[:, mb : mb + 1])
                st["agg"] = agg

            def emit_fold(st):
                # preattn = [x, agg] @ fold  -> [8, F]
                agg, xr = st["agg"], st["xr"]
                pf = psum8.tile([H, F], F32)
                nc.tensor.matmul(pf[:, :], wfxs[:, :], xr[:, :], start=True, stop=False)
                for kb in range(NB):
                    nc.tensor.matmul(
                        pf[:, :], wfas[:, kb, :], agg[:, kb, :],
                        start=False, stop=(kb == NB - 1),
                    )
                oc = io.tile([H, F], F32, tag="oc")
                nc.vector.tensor_copy(oc, pf)
                nc.sync.dma_start(out=out[:, st["cs"]], in_=oc)

            prev_y = None
            p1 = None
            for c in range(nchunk):
                cs = slice(c * F, (c + 1) * F)

                xf = io.tile([128, F], F32)
                nc.sync.dma_start(out=xf, in_=xT[:, cs])
                xr = io.tile([128, F], F32R)
                nc.vector.tensor_copy(xr, xf)
                gt = io.tile([128, F], F32)
                nc.sync.dma_start(out=gt, in_=gam[0:1, cs].partition_broadcast(128))
                it = io.tile([128, F], F32)
                nc.sync.dma_start(out=it, in_=invc[0:1, cs].partition_broadcast(128))

                # psi layer 1: [128, F] -> [512, F]
                h1 = acts.tile([128, NB, F], F32R)
                for mb in range(NB):
                    ps = psum.tile([128, F], F32, tag="ps")
                    nc.tensor.matmul(
                        ps[:, :], w1s[:, mb * 128 : (mb + 1) * 128], xr[:, :],
                        start=True, stop=True,
                    )
                    nc.scalar.activation(h1[:, mb, :], ps, RELU, bias=b1s[:, mb : mb + 1])

                # psi layer 2
                h2 = acts.tile([128, NB, F], F32R)
                for mb in range(NB):
                    ps = psum.tile([128, F], F32, tag="ps")
                    for kb in range(NB):
                        nc.tensor.matmul(
                            ps[:, :], w2s[:, kb, mb * 128 : (mb + 1) * 128], h1[:, kb, :],
                            start=(kb == 0), stop=(kb == NB - 1),
                        )
                    nc.scalar.activation(h2[:, mb, :], ps, RELU, bias=b2s[:, mb : mb + 1])

                # prev chunk's rho here: the scan(c-1) got L1+L2 of PE time
                if p1 is not None:
                    emit_rho(p1)

                # psi latent projection
                enc = acts.tile([128, NB, F], F32R)
                for mb in range(NB):
                    ps = psum.tile([128, F], F32, tag="ps")
                    for kb in range(NB):
                        nc.tensor.matmul(
                            ps[:, :], w3s[:, kb, mb * 128 : (mb + 1) * 128], h2[:, kb, :],
                            start=(kb == 0), stop=(kb == NB - 1),
                        )
                    nc.scalar.activation(enc[:, mb, :], ps, RELU, bias=b3s[:, mb : mb + 1])

                # segmented cumulative sum along tokens (resets where gamma=0)
                y = scan.tile([128, NB, F], F32R)
                for b in range(NB):
                    init = zcarry[:, b : b + 1] if prev_y is None else prev_y[:, b, F - 1 : F]
                    nc.vector.tensor_tensor_scan(
                        out=y[:, b, :], data0=gt[:, :], data1=enc[:, b, :].bitcast(F32),
                        initial=init, op0=mybir.AluOpType.mult, op1=mybir.AluOpType.add,
                    )
                prev_y = y

                # prev chunk's fold here: its agg chain got L3 of PE time
                if p1 is not None:
                    emit_fold(p1)
                p1 = {"y": y, "it": it, "xr": xr, "cs": cs}
            emit_rho(p1)
            emit_fold(p1)

    nc.compile()
    return nc


def kernel(inputs, segment_ids, W1, b1, W2, b2, W3, b3, Wr, br, W_k, W_q):
    inputs = np.ascontiguousarray(inputs, np.float32)
    segment_ids = np.asarray(segment_ids)
    n = inputs.shape[0]

    # fold the per-head attention weights: preattn = [inputs, agg] @ wfold
    wfold = np.einsum(
        "chd,hd->ch",
        np.asarray(W_k, np.float64).reshape(D_IN + LAT, H, DPD),
        np.asarray(W_q, np.float64),
    ) / math.sqrt(DPD)
    wfold = wfold.astype(np.float32)
    wfx_np = np.ascontiguousarray(wfold[:D_IN])
    wfa_np = np.ascontiguousarray(wfold[D_IN:])

    # shard at segment boundaries nearest the N/8 grid
    is_start = np.empty(n, bool)
    is_start[0] = True
    is_start[1:] = segment_ids[1:] != segment_ids[:-1]
    starts = np.flatnonzero(is_start)
    bounds = [0]
    for c in range(1, NCORES):
        ideal = c * (n // NCORES)
        i = np.searchsorted(starts, ideal)
        cand = int(starts[i]) if i < len(starts) else n
        if i > 0 and abs(int(starts[i - 1]) - ideal) <= abs(cand - ideal):
            cand = int(starts[i - 1])
        bounds.append(max(cand, bounds[-1]))
    bounds.append(n)

    maxlen = max(bounds[c + 1] - bounds[c] for c in range(NCORES))
    npad = ((maxlen + F - 1) // F) * F

    if npad not in _BUILD_CACHE:
        _BUILD_CACHE[npad] = _build(npad)
    nc = _BUILD_CACHE[npad]

    bias_np = np.stack(
        [np.asarray(v, np.float32).reshape(NB, 128).T for v in (b1, b2, b3, br)], 1
    ).reshape(128, 4 * NB)
    bias_np = np.ascontiguousarray(bias_np)

    common = {
        "w1": np.ascontiguousarray(W1, np.float32),
        "w2": np.ascontiguousarray(W2, np.float32),
        "w3": np.ascontiguousarray(W3, np.float32),
        "wr": np.ascontiguousarray(Wr, np.float32),
        "wfx": wfx_np,
        "wfa": wfa_np,
        "bias": bias_np,
    }

    in_maps = []
    for c in range(NCORES):
        lo, hi = bounds[c], bounds[c + 1]
        L = hi - lo
        xT = np.zeros((D_IN, npad), np.float32)
        xT[:, :L] = inputs[lo:hi].T
        g = np.ones((1, npad), np.float32)
        g[0, :L] = 1.0 - is_start[lo:hi]
        g[0, 0] = 0.0
        if L < npad:
            g[0, L] = 0.0
        idx = np.arange(L)
        st = np.maximum.accumulate(np.where(is_start[lo:hi], idx, 0))
        iv = np.ones((1, npad), np.float32)
        iv[0, :L] = 1.0 / (idx - st + 1.0)
        in_maps.append({"xT": xT, "gam": g, "invc": iv, **common})

    global _LAST_IN_MAPS
    _LAST_IN_MAPS = in_maps

    res = run_bass_kernel_spmd(nc, in_maps, core_ids=list(range(NCORES)))

    full = np.empty((n, H), np.float32)
    for c in range(NCORES):
        lo, hi = bounds[c], bounds[c + 1]
        full[lo:hi] = res.results[c]["out"][:, : hi - lo].T
    return full
